# revision 11
# baseline (speedup 1.0000x reference)
"""Expert-parallel MoE MLP kernel for TRN2 (8 NeuronCores).

Reference computation (all experts, dense routing):
    hidden = einsum("bnd,edh->benh", x, w1); hidden = gelu(hidden)
    out    = einsum("benh,ehd->bnde", hidden, w2)        # [b, n, d4, e]

Sharding: expert-parallel, 2 experts per core (16 experts / 8 cores); x is
replicated. Each core computes, for its experts e:
    hT[e] = gelu(W1[e].T @ X.T)        # [h, tok] layout, h on partitions
    outT[e] = W2[e].T @ hT[e]          # [d4, tok] layout
which keeps the contraction dim on SBUF partitions for both matmuls with no
on-device transposes. This problem is PE-bound: 5.37e9 MACs/core at 16384
MACs/cycle / 2.4 GHz = 136.5 us floor (fp8 DoubleRow is ~1.44x at best but
e4m3 quantization alone costs 4-5% rel err vs the 2e-2 budget, so bf16 it is).

Schedule notes:
- bf16 everywhere halves DMA vs fp32r at the same matmul rate; output is
  written bf16 and upcast on the host (total rel err ~3.8e-3).
- A chain of tiny warm-up matmuls at t=0 ramps the PE HAM clock gate
  (1.2 -> 2.4 GHz) and covers the DMA staging head.
- dma_start descriptor generation costs ~0.6us each on the issuing
  sequencer, so inputs are a few big DMAs on SP while output DMAs are
  issued from the Activation engine (also HW-DGE capable) to keep them
  off SP's queue: a front-loaded input stream would otherwise block
  output DMAs, exhaust the output tile pools, and stall the PE.
"""

import sys

import numpy as np

for _p in ("/opt/trn_rl_repo", "/root/.axon_site/_ro/trn_rl_repo"):
    if _p not in sys.path:
        sys.path.append(_p)

import ml_dtypes

import concourse.bacc as bacc
import concourse.mybir as mybir
import concourse.tile as tile
from concourse.bass_utils import run_bass_kernel_spmd

F32 = mybir.dt.float32
BF16 = mybir.dt.bfloat16

N_CORES = 8
E = 16                 # total experts
E_LOC = E // N_CORES   # experts per core
D = 512                # model dim (contraction of mm1)
H = 512                # hidden dim (contraction of mm2)
D4 = 128               # output dim per expert
NTOK = 4 * 2048        # tokens
TT = 512               # token tile (matmul moving free dim)
P = 128
N_DT = D // P          # 4 k-tiles of mm1
N_HT = H // P          # 4 k-tiles of mm2
WARMUP_MMS = 48        # PE HAM ramp filler (N=64 matmuls on a zeroed tile)


def _build_program():
    nc = bacc.Bacc("TRN2", target_bir_lowering=False, debug=False)
    # DRAM layouts are pre-swizzled on the host so every DMA lands in SBUF
    # in matmul-ready [partition, free...] form with contiguous rows.
    xT = nc.declare_dram_parameter("xT", [P, N_DT, NTOK], BF16, isOutput=False)
    w1 = nc.declare_dram_parameter("w1", [P, E_LOC, N_DT, H], BF16, isOutput=False)
    w2 = nc.declare_dram_parameter("w2", [P, E_LOC, N_HT, D4], BF16, isOutput=False)
    outT = nc.declare_dram_parameter("outT", [D4, E_LOC, NTOK], BF16, isOutput=True)

    gelu = mybir.ActivationFunctionType.Gelu

    with tile.TileContext(nc) as tc:
        with (
            tc.tile_pool(name="wpool", bufs=1) as wpool,
            tc.tile_pool(name="hpool", bufs=3) as hpool,
            tc.tile_pool(name="opool", bufs=4) as opool,
            tc.tile_pool(name="ps1p", bufs=4, space="PSUM") as ps1p,
            tc.tile_pool(name="ps2p", bufs=3, space="PSUM") as ps2p,
        ):
            # --- PE warm-up: small matmuls on a zeroed tile, no DMA deps.
            # memset on DVE so the ACT engine's one-time Gelu table load can
            # happen concurrently (and isn't preceded by a Copy-table load).
            warm = wpool.tile([P, 64], BF16, name="warm", tag="warm")
            nc.vector.memset(warm, 0.0)
            ps_w = ps1p.tile([P, TT], F32, name="ps_w", tag="ps1")
            for _ in range(WARMUP_MMS):
                nc.tensor.matmul(ps_w[:64, :64], warm[:, :64], warm, start=True, stop=True)

            # --- Weights + all of x resident in SBUF (bf16: 74KB/partition).
            w1_sb = wpool.tile([P, E_LOC, N_DT, H], BF16, name="w1_sb", tag="w1")
            w2_sb = wpool.tile([P, E_LOC, N_HT, D4], BF16, name="w2_sb", tag="w2")
            x_sb = wpool.tile([P, N_DT, NTOK], BF16, name="x_sb", tag="x")

            # Input stream on SP, few big DMAs, first-needed first.
            nc.sync.dma_start(w1_sb[:, 0], w1[:, 0])
            tok0 = slice(0, TT)
            for dt in range(N_DT):
                nc.sync.dma_start(x_sb[:, dt, tok0], xT[:, dt, tok0])
            nc.sync.dma_start(w2_sb[:, 0], w2[:, 0])
            for e in range(1, E_LOC):
                nc.sync.dma_start(w1_sb[:, e], w1[:, e])
                nc.sync.dma_start(w2_sb[:, e], w2[:, e])
            for t0, t1 in ((1, 2), (2, 3), (3, 4), (4, 8), (8, 12), (12, 16)):
                tok = slice(t0 * TT, t1 * TT)
                nc.sync.dma_start(x_sb[:, :, tok], xT[:, :, tok])

            for t in range(NTOK // TT):
                tok = slice(t * TT, (t + 1) * TT)
                hT_tiles = []
                for e in range(E_LOC):
                    hT_sb = hpool.tile([P, N_HT, TT], BF16, name="hT_sb", tag="h")
                    for ht in range(N_HT):
                        ps1 = ps1p.tile([P, TT], F32, name="ps1", tag="ps1")
                        for dt in range(N_DT):
                            nc.tensor.matmul(
                                ps1,
                                w1_sb[:, e, dt, ht * P : (ht + 1) * P],
                                x_sb[:, dt, tok],
                                start=(dt == 0),
                                stop=(dt == N_DT - 1),
                            )
                        nc.scalar.activation(hT_sb[:, ht, :], ps1, gelu)
                    hT_tiles.append(hT_sb)
                o_sb = opool.tile([P, E_LOC, TT], BF16, name="o_sb", tag="o")
                for e in range(E_LOC):
                    ps2 = ps2p.tile([P, TT], F32, name="ps2", tag="ps2")
                    for ht in range(N_HT):
                        nc.tensor.matmul(
                            ps2,
                            w2_sb[:, e, ht, :],
                            hT_tiles[e][:, ht, :],
                            start=(ht == 0),
                            stop=(ht == N_HT - 1),
                        )
                    nc.vector.tensor_copy(o_sb[:, e, :], ps2)
                    if t == NTOK // TT - 1:
                        # Last tile: split per expert so the final (small)
                        # DMA starts as soon as its own cast is done.
                        nc.scalar.dma_start(outT[:, e, tok], o_sb[:, e, :])
                if t < NTOK // TT - 1:
                    # Output DMA from the Activation engine's HW-DGE queue
                    # (SP-issued DMAs mid-kernel congest the semaphore path).
                    nc.scalar.dma_start(outT[:, :, tok], o_sb)

    nc.finalize()
    return nc


_NC = None


def _get_program():
    global _NC
    if _NC is None:
        _NC = _build_program()
    return _NC


def _prepare_in_maps(x: np.ndarray, w1: np.ndarray, w2: np.ndarray):
    """Host-side swizzle + bf16 cast into per-core input maps."""
    # xT[p, dt, n] = x[n, dt*128 + p]
    xT = (
        x.reshape(NTOK, N_DT, P)
        .transpose(2, 1, 0)
        .astype(ml_dtypes.bfloat16)
    )
    xT = np.ascontiguousarray(xT)
    in_maps = []
    for c in range(N_CORES):
        w1c = w1[c * E_LOC : (c + 1) * E_LOC]  # [e, d, h]
        w2c = w2[c * E_LOC : (c + 1) * E_LOC]  # [e, h, d4]
        # w1_dr[p, e, dt, h] = w1c[e, dt*128+p, h]
        w1d = np.ascontiguousarray(
            w1c.reshape(E_LOC, N_DT, P, H).transpose(2, 0, 1, 3)
        ).astype(ml_dtypes.bfloat16)
        w2d = np.ascontiguousarray(
            w2c.reshape(E_LOC, N_HT, P, D4).transpose(2, 0, 1, 3)
        ).astype(ml_dtypes.bfloat16)
        in_maps.append({"xT": xT, "w1": w1d, "w2": w2d})
    return in_maps


def kernel(x: np.ndarray, w1: np.ndarray, w2: np.ndarray, **_) -> np.ndarray:
    """Full inputs in, full output out; expert-parallel across 8 NeuronCores."""
    nc = _get_program()
    in_maps = _prepare_in_maps(x, w1, w2)
    res = run_bass_kernel_spmd(nc, in_maps, list(range(N_CORES)))

    # res outT: [d4, e_loc, tok] per core -> out[n, d4, e] with e = c*E_LOC+el
    full = np.stack([res.results[c]["outT"] for c in range(N_CORES)], axis=0)
    out = full.transpose(3, 1, 0, 2).astype(np.float32)  # [tok, d4, core, e_loc]
    return np.ascontiguousarray(out.reshape(4, 2048, D4, E))


# revision 12
# speedup vs baseline: 1.1905x; 1.1905x over previous
"""Expert-parallel MoE MLP kernel for TRN2 (8 NeuronCores).

Reference computation (all experts, dense routing):
    hidden = einsum("bnd,edh->benh", x, w1); hidden = gelu(hidden)
    out    = einsum("benh,ehd->bnde", hidden, w2)        # [b, n, d4, e]

Sharding: expert-parallel, 2 experts per core (16 experts / 8 cores); x is
replicated. Each core computes, for its experts e:
    hT[e] = gelu(W1[e].T @ X.T)        # [h, tok] layout, h on partitions
    outT[e] = W2[e].T @ hT[e]          # [d4, tok] layout
which keeps the contraction dim on SBUF partitions for both matmuls with no
on-device transposes. This problem is PE-bound: 5.37e9 MACs/core at 16384
MACs/cycle / 2.4 GHz = 136.5 us floor (fp8 DoubleRow is ~1.44x at best but
e4m3 quantization alone costs 4-5% rel err vs the 2e-2 budget, so bf16 it is).

Schedule notes:
- bf16 everywhere halves DMA vs fp32r at the same matmul rate; output is
  written bf16 and upcast on the host (total rel err ~3.8e-3).
- A chain of tiny warm-up matmuls at t=0 ramps the PE HAM clock gate
  (1.2 -> 2.4 GHz) and covers the DMA staging head.
- dma_start descriptor generation costs ~0.6us each on the issuing
  sequencer, so inputs are a few big DMAs on SP while output DMAs are
  issued from the Activation engine (also HW-DGE capable) to keep them
  off SP's queue: a front-loaded input stream would otherwise block
  output DMAs, exhaust the output tile pools, and stall the PE.
"""

import sys

import numpy as np

for _p in ("/opt/trn_rl_repo", "/root/.axon_site/_ro/trn_rl_repo"):
    if _p not in sys.path:
        sys.path.append(_p)

import ml_dtypes

import concourse.bacc as bacc
import concourse.mybir as mybir
import concourse.tile as tile
from concourse.bass_utils import run_bass_kernel_spmd

F32 = mybir.dt.float32
BF16 = mybir.dt.bfloat16

N_CORES = 8
E = 16                 # total experts
E_LOC = E // N_CORES   # experts per core
D = 512                # model dim (contraction of mm1)
H = 512                # hidden dim (contraction of mm2)
D4 = 128               # output dim per expert
NTOK = 4 * 2048        # tokens
TT = 512               # token tile (matmul moving free dim)
P = 128
N_DT = D // P          # 4 k-tiles of mm1
N_HT = H // P          # 4 k-tiles of mm2
WARMUP_MMS = 48        # PE HAM ramp filler (N=64 matmuls on a zeroed tile)


def _build_program():
    nc = bacc.Bacc("TRN2", target_bir_lowering=False, debug=False)
    # DRAM layouts are pre-swizzled on the host so every DMA lands in SBUF
    # in matmul-ready [partition, free...] form with contiguous rows.
    xT = nc.declare_dram_parameter("xT", [P, N_DT, NTOK], BF16, isOutput=False)
    w1 = nc.declare_dram_parameter("w1", [P, E_LOC, N_DT, H], BF16, isOutput=False)
    w2 = nc.declare_dram_parameter("w2", [P, E_LOC, N_HT, D4], BF16, isOutput=False)
    outT = nc.declare_dram_parameter("outT", [D4, E_LOC, NTOK], BF16, isOutput=True)

    gelu = mybir.ActivationFunctionType.Gelu

    with tile.TileContext(nc) as tc:
        with (
            tc.tile_pool(name="wpool", bufs=1) as wpool,
            tc.tile_pool(name="hpool", bufs=3) as hpool,
            tc.tile_pool(name="opool", bufs=4) as opool,
            tc.tile_pool(name="ps1p", bufs=4, space="PSUM") as ps1p,
            tc.tile_pool(name="ps2p", bufs=3, space="PSUM") as ps2p,
        ):
            # --- PE warm-up: small matmuls on a zeroed tile, no DMA deps.
            # memset on DVE so the ACT engine's one-time Gelu table load can
            # happen concurrently (and isn't preceded by a Copy-table load).
            warm = wpool.tile([P, 64], BF16, name="warm", tag="warm")
            nc.vector.memset(warm, 0.0)
            ps_w = ps1p.tile([P, TT], F32, name="ps_w", tag="ps1")
            for _ in range(WARMUP_MMS):
                nc.tensor.matmul(ps_w[:64, :64], warm[:, :64], warm, start=True, stop=True)

            # --- Weights + all of x resident in SBUF (bf16: 74KB/partition).
            w1_sb = wpool.tile([P, E_LOC, N_DT, H], BF16, name="w1_sb", tag="w1")
            w2_sb = wpool.tile([P, E_LOC, N_HT, D4], BF16, name="w2_sb", tag="w2")
            x_sb = wpool.tile([P, N_DT, NTOK], BF16, name="x_sb", tag="x")

            # Input stream on SP, few big DMAs, first-needed first.
            nc.sync.dma_start(w1_sb[:, 0], w1[:, 0])
            tok0 = slice(0, TT)
            for dt in range(N_DT):
                nc.sync.dma_start(x_sb[:, dt, tok0], xT[:, dt, tok0])
            nc.sync.dma_start(w2_sb[:, 0], w2[:, 0])
            for e in range(1, E_LOC):
                nc.sync.dma_start(w1_sb[:, e], w1[:, e])
                nc.sync.dma_start(w2_sb[:, e], w2[:, e])
            for t0, t1 in ((1, 2), (2, 3), (3, 4), (4, 8), (8, 12), (12, 16)):
                tok = slice(t0 * TT, t1 * TT)
                nc.sync.dma_start(x_sb[:, :, tok], xT[:, :, tok])

            for t in range(NTOK // TT):
                tok = slice(t * TT, (t + 1) * TT)
                hT_tiles = []
                for e in range(E_LOC):
                    hT_sb = hpool.tile([P, N_HT, TT], BF16, name="hT_sb", tag="h")
                    for ht in range(N_HT):
                        ps1 = ps1p.tile([P, TT], F32, name="ps1", tag="ps1")
                        for dt in range(N_DT):
                            nc.tensor.matmul(
                                ps1,
                                w1_sb[:, e, dt, ht * P : (ht + 1) * P],
                                x_sb[:, dt, tok],
                                start=(dt == 0),
                                stop=(dt == N_DT - 1),
                            )
                        nc.scalar.activation(hT_sb[:, ht, :], ps1, gelu)
                    hT_tiles.append(hT_sb)
                o_sb = opool.tile([P, E_LOC, TT], BF16, name="o_sb", tag="o")
                for e in range(E_LOC):
                    ps2 = ps2p.tile([P, TT], F32, name="ps2", tag="ps2")
                    for ht in range(N_HT):
                        nc.tensor.matmul(
                            ps2,
                            w2_sb[:, e, ht, :],
                            hT_tiles[e][:, ht, :],
                            start=(ht == 0),
                            stop=(ht == N_HT - 1),
                        )
                    nc.vector.tensor_copy(o_sb[:, e, :], ps2)
                # Output DMA from the Activation engine's HW-DGE queue
                # (SP-issued DMAs mid-kernel congest the semaphore path).
                nc.scalar.dma_start(outT[:, :, tok], o_sb)

    nc.finalize()
    return nc


_NC = None


def _get_program():
    global _NC
    if _NC is None:
        _NC = _build_program()
    return _NC


def _prepare_in_maps(x: np.ndarray, w1: np.ndarray, w2: np.ndarray):
    """Host-side swizzle + bf16 cast into per-core input maps."""
    # xT[p, dt, n] = x[n, dt*128 + p]
    xT = (
        x.reshape(NTOK, N_DT, P)
        .transpose(2, 1, 0)
        .astype(ml_dtypes.bfloat16)
    )
    xT = np.ascontiguousarray(xT)
    in_maps = []
    for c in range(N_CORES):
        w1c = w1[c * E_LOC : (c + 1) * E_LOC]  # [e, d, h]
        w2c = w2[c * E_LOC : (c + 1) * E_LOC]  # [e, h, d4]
        # w1_dr[p, e, dt, h] = w1c[e, dt*128+p, h]
        w1d = np.ascontiguousarray(
            w1c.reshape(E_LOC, N_DT, P, H).transpose(2, 0, 1, 3)
        ).astype(ml_dtypes.bfloat16)
        w2d = np.ascontiguousarray(
            w2c.reshape(E_LOC, N_HT, P, D4).transpose(2, 0, 1, 3)
        ).astype(ml_dtypes.bfloat16)
        in_maps.append({"xT": xT, "w1": w1d, "w2": w2d})
    return in_maps


def kernel(x: np.ndarray, w1: np.ndarray, w2: np.ndarray, **_) -> np.ndarray:
    """Full inputs in, full output out; expert-parallel across 8 NeuronCores."""
    nc = _get_program()
    in_maps = _prepare_in_maps(x, w1, w2)
    res = run_bass_kernel_spmd(nc, in_maps, list(range(N_CORES)))

    # res outT: [d4, e_loc, tok] per core -> out[n, d4, e] with e = c*E_LOC+el
    full = np.stack([res.results[c]["outT"] for c in range(N_CORES)], axis=0)
    out = full.transpose(3, 1, 0, 2).astype(np.float32)  # [tok, d4, core, e_loc]
    return np.ascontiguousarray(out.reshape(4, 2048, D4, E))


# revision 14
# speedup vs baseline: 1.1994x; 1.0074x over previous
"""Expert-parallel MoE MLP kernel for TRN2 (8 NeuronCores).

Reference computation (all experts, dense routing):
    hidden = einsum("bnd,edh->benh", x, w1); hidden = gelu(hidden)
    out    = einsum("benh,ehd->bnde", hidden, w2)        # [b, n, d4, e]

Sharding: expert-parallel, 2 experts per core (16 experts / 8 cores); x is
replicated. Each core computes, for its experts e:
    hT[e] = gelu(W1[e].T @ X.T)        # [h, tok] layout, h on partitions
    outT[e] = W2[e].T @ hT[e]          # [d4, tok] layout
which keeps the contraction dim on SBUF partitions for both matmuls with no
on-device transposes. This problem is PE-bound: 5.37e9 MACs/core at 16384
MACs/cycle / 2.4 GHz = 136.5 us floor (fp8 DoubleRow is ~1.44x at best but
e4m3 quantization alone costs 4-5% rel err vs the 2e-2 budget, so bf16 it is).

Schedule notes:
- bf16 everywhere halves DMA vs fp32r at the same matmul rate; output is
  written bf16 and upcast on the host (total rel err ~3.8e-3).
- A chain of tiny warm-up matmuls at t=0 ramps the PE HAM clock gate
  (1.2 -> 2.4 GHz) and covers the DMA staging head.
- dma_start descriptor generation costs ~0.6us each on the issuing
  sequencer, so inputs are a few big DMAs on SP while output DMAs are
  issued from the Activation engine (also HW-DGE capable) to keep them
  off SP's queue: a front-loaded input stream would otherwise block
  output DMAs, exhaust the output tile pools, and stall the PE.
"""

import sys

import numpy as np

for _p in ("/opt/trn_rl_repo", "/root/.axon_site/_ro/trn_rl_repo"):
    if _p not in sys.path:
        sys.path.append(_p)

import ml_dtypes

import concourse.bacc as bacc
import concourse.mybir as mybir
import concourse.tile as tile
from concourse.bass_utils import run_bass_kernel_spmd

F32 = mybir.dt.float32
BF16 = mybir.dt.bfloat16

N_CORES = 8
E = 16                 # total experts
E_LOC = E // N_CORES   # experts per core
D = 512                # model dim (contraction of mm1)
H = 512                # hidden dim (contraction of mm2)
D4 = 128               # output dim per expert
NTOK = 4 * 2048        # tokens
TT = 512               # token tile (matmul moving free dim)
P = 128
N_DT = D // P          # 4 k-tiles of mm1
N_HT = H // P          # 4 k-tiles of mm2
WARMUP_MMS = 80        # PE HAM ramp filler (N=64 matmuls on a zeroed tile)


def _build_program():
    nc = bacc.Bacc("TRN2", target_bir_lowering=False, debug=False)
    # DRAM layouts are pre-swizzled on the host so every DMA lands in SBUF
    # in matmul-ready [partition, free...] form with contiguous rows.
    xT = nc.declare_dram_parameter("xT", [P, N_DT, NTOK], BF16, isOutput=False)
    w1 = nc.declare_dram_parameter("w1", [P, E_LOC, N_DT, H], BF16, isOutput=False)
    w2 = nc.declare_dram_parameter("w2", [P, E_LOC, N_HT, D4], BF16, isOutput=False)
    outT = nc.declare_dram_parameter("outT", [D4, E_LOC, NTOK], BF16, isOutput=True)

    gelu = mybir.ActivationFunctionType.Gelu

    with tile.TileContext(nc) as tc:
        with (
            tc.tile_pool(name="wpool", bufs=1) as wpool,
            tc.tile_pool(name="hpool", bufs=3) as hpool,
            tc.tile_pool(name="opool", bufs=4) as opool,
            tc.tile_pool(name="ps1p", bufs=4, space="PSUM") as ps1p,
            tc.tile_pool(name="ps2p", bufs=3, space="PSUM") as ps2p,
        ):
            # --- PE warm-up: small matmuls on a zeroed tile, no DMA deps.
            # memset on DVE so the ACT engine's one-time Gelu table load can
            # happen concurrently (and isn't preceded by a Copy-table load).
            warm = wpool.tile([P, 64], BF16, name="warm", tag="warm")
            nc.vector.memset(warm, 0.0)
            ps_w = ps1p.tile([P, TT], F32, name="ps_w", tag="ps1")
            for _ in range(WARMUP_MMS):
                nc.tensor.matmul(ps_w[:64, :64], warm[:, :64], warm, start=True, stop=True)

            # --- Weights + all of x resident in SBUF (bf16: 74KB/partition).
            w1_sb = wpool.tile([P, E_LOC, N_DT, H], BF16, name="w1_sb", tag="w1")
            w2_sb = wpool.tile([P, E_LOC, N_HT, D4], BF16, name="w2_sb", tag="w2")
            x_sb = wpool.tile([P, N_DT, NTOK], BF16, name="x_sb", tag="x")

            # Input stream on SP, few big DMAs, first-needed first.
            nc.sync.dma_start(w1_sb[:, 0], w1[:, 0])
            tok0 = slice(0, TT)
            for dt in range(N_DT):
                nc.sync.dma_start(x_sb[:, dt, tok0], xT[:, dt, tok0])
            nc.sync.dma_start(w2_sb[:, 0], w2[:, 0])
            for e in range(1, E_LOC):
                nc.sync.dma_start(w1_sb[:, e], w1[:, e])
                nc.sync.dma_start(w2_sb[:, e], w2[:, e])
            for t0, t1 in ((1, 2), (2, 3), (3, 4), (4, 8), (8, 12), (12, 16)):
                tok = slice(t0 * TT, t1 * TT)
                nc.sync.dma_start(x_sb[:, :, tok], xT[:, :, tok])

            for t in range(NTOK // TT):
                tok = slice(t * TT, (t + 1) * TT)
                hT_tiles = []
                for e in range(E_LOC):
                    hT_sb = hpool.tile([P, N_HT, TT], BF16, name="hT_sb", tag="h")
                    for ht in range(N_HT):
                        ps1 = ps1p.tile([P, TT], F32, name="ps1", tag="ps1")
                        for dt in range(N_DT):
                            nc.tensor.matmul(
                                ps1,
                                w1_sb[:, e, dt, ht * P : (ht + 1) * P],
                                x_sb[:, dt, tok],
                                start=(dt == 0),
                                stop=(dt == N_DT - 1),
                            )
                        nc.scalar.activation(hT_sb[:, ht, :], ps1, gelu)
                    hT_tiles.append(hT_sb)
                # The last token tile runs mm2/cast/DMA in two 256-token
                # halves so the final output DMA starts ~2us earlier.
                halves = 2 if t == NTOK // TT - 1 else 1
                hw = TT // halves
                for h in range(halves):
                    hsl = slice(h * hw, (h + 1) * hw)
                    o_sb = opool.tile([P, E_LOC, hw], BF16, name="o_sb", tag="o")
                    for e in range(E_LOC):
                        ps2 = ps2p.tile([P, hw], F32, name="ps2", tag="ps2")
                        for ht in range(N_HT):
                            nc.tensor.matmul(
                                ps2,
                                w2_sb[:, e, ht, :],
                                hT_tiles[e][:, ht, hsl],
                                start=(ht == 0),
                                stop=(ht == N_HT - 1),
                            )
                        nc.vector.tensor_copy(o_sb[:, e, :], ps2)
                    # Output DMA from the Activation engine's HW-DGE queue
                    # (SP-issued DMAs mid-kernel congest the semaphore path).
                    htok = slice(t * TT + h * hw, t * TT + (h + 1) * hw)
                    nc.scalar.dma_start(outT[:, :, htok], o_sb)

    nc.finalize()
    return nc


_NC = None


def _get_program():
    global _NC
    if _NC is None:
        _NC = _build_program()
    return _NC


def _prepare_in_maps(x: np.ndarray, w1: np.ndarray, w2: np.ndarray):
    """Host-side swizzle + bf16 cast into per-core input maps."""
    # xT[p, dt, n] = x[n, dt*128 + p]
    xT = (
        x.reshape(NTOK, N_DT, P)
        .transpose(2, 1, 0)
        .astype(ml_dtypes.bfloat16)
    )
    xT = np.ascontiguousarray(xT)
    in_maps = []
    for c in range(N_CORES):
        w1c = w1[c * E_LOC : (c + 1) * E_LOC]  # [e, d, h]
        w2c = w2[c * E_LOC : (c + 1) * E_LOC]  # [e, h, d4]
        # w1_dr[p, e, dt, h] = w1c[e, dt*128+p, h]
        w1d = np.ascontiguousarray(
            w1c.reshape(E_LOC, N_DT, P, H).transpose(2, 0, 1, 3)
        ).astype(ml_dtypes.bfloat16)
        w2d = np.ascontiguousarray(
            w2c.reshape(E_LOC, N_HT, P, D4).transpose(2, 0, 1, 3)
        ).astype(ml_dtypes.bfloat16)
        in_maps.append({"xT": xT, "w1": w1d, "w2": w2d})
    return in_maps


def kernel(x: np.ndarray, w1: np.ndarray, w2: np.ndarray, **_) -> np.ndarray:
    """Full inputs in, full output out; expert-parallel across 8 NeuronCores."""
    nc = _get_program()
    in_maps = _prepare_in_maps(x, w1, w2)
    res = run_bass_kernel_spmd(nc, in_maps, list(range(N_CORES)))

    # res outT: [d4, e_loc, tok] per core -> out[n, d4, e] with e = c*E_LOC+el
    full = np.stack([res.results[c]["outT"] for c in range(N_CORES)], axis=0)
    out = full.transpose(3, 1, 0, 2).astype(np.float32)  # [tok, d4, core, e_loc]
    return np.ascontiguousarray(out.reshape(4, 2048, D4, E))


# revision 17
# speedup vs baseline: 1.2074x; 1.0067x over previous
"""Expert-parallel MoE MLP kernel for TRN2 (8 NeuronCores).

Reference computation (all experts, dense routing):
    hidden = einsum("bnd,edh->benh", x, w1); hidden = gelu(hidden)
    out    = einsum("benh,ehd->bnde", hidden, w2)        # [b, n, d4, e]

Sharding: expert-parallel, 2 experts per core (16 experts / 8 cores); x is
replicated. Each core computes, for its experts e:
    hT[e] = gelu(W1[e].T @ X.T)        # [h, tok] layout, h on partitions
    outT[e] = W2[e].T @ hT[e]          # [d4, tok] layout
which keeps the contraction dim on SBUF partitions for both matmuls with no
on-device transposes. This problem is PE-bound: 5.37e9 MACs/core at 16384
MACs/cycle / 2.4 GHz = 136.5 us floor (fp8 DoubleRow is ~1.44x at best but
e4m3 quantization alone costs 4-5% rel err vs the 2e-2 budget, so bf16 it is).

Schedule notes:
- bf16 everywhere halves DMA vs fp32r at the same matmul rate; output is
  written bf16 and upcast on the host (total rel err ~3.8e-3).
- A chain of tiny warm-up matmuls at t=0 ramps the PE HAM clock gate
  (1.2 -> 2.4 GHz) and covers the DMA staging head.
- dma_start descriptor generation costs ~0.6us each on the issuing
  sequencer, so inputs are a few big DMAs on SP while output DMAs are
  issued from the Activation engine (also HW-DGE capable) to keep them
  off SP's queue: a front-loaded input stream would otherwise block
  output DMAs, exhaust the output tile pools, and stall the PE.
"""

import sys

import numpy as np

for _p in ("/opt/trn_rl_repo", "/root/.axon_site/_ro/trn_rl_repo"):
    if _p not in sys.path:
        sys.path.append(_p)

import ml_dtypes

import concourse.bacc as bacc
import concourse.mybir as mybir
import concourse.tile as tile
from concourse.bass_utils import run_bass_kernel_spmd

F32 = mybir.dt.float32
BF16 = mybir.dt.bfloat16

N_CORES = 8
E = 16                 # total experts
E_LOC = E // N_CORES   # experts per core
D = 512                # model dim (contraction of mm1)
H = 512                # hidden dim (contraction of mm2)
D4 = 128               # output dim per expert
NTOK = 4 * 2048        # tokens
TT = 512               # token tile (matmul moving free dim)
P = 128
N_DT = D // P          # 4 k-tiles of mm1
N_HT = H // P          # 4 k-tiles of mm2
WARMUP_MMS = 48        # PE HAM ramp filler (N=64 matmuls on a zeroed tile)
WARMUP_BIG_MMS = 6     # N=512 tail of the warm-up chain (solid busy windows)


def _build_program():
    nc = bacc.Bacc("TRN2", target_bir_lowering=False, debug=False)
    # DRAM layouts are pre-swizzled on the host so every DMA lands in SBUF
    # in matmul-ready [partition, free...] form with contiguous rows.
    xT = nc.declare_dram_parameter("xT", [P, N_DT, NTOK], BF16, isOutput=False)
    w1 = nc.declare_dram_parameter("w1", [P, E_LOC, N_DT, H], BF16, isOutput=False)
    w2 = nc.declare_dram_parameter("w2", [P, E_LOC, N_HT, D4], BF16, isOutput=False)
    outT = nc.declare_dram_parameter("outT", [D4, E_LOC, NTOK], BF16, isOutput=True)

    gelu = mybir.ActivationFunctionType.Gelu

    with tile.TileContext(nc) as tc:
        with (
            tc.tile_pool(name="wpool", bufs=1) as wpool,
            tc.tile_pool(name="hpool", bufs=3) as hpool,
            tc.tile_pool(name="opool", bufs=4) as opool,
            tc.tile_pool(name="ps1p", bufs=4, space="PSUM") as ps1p,
            tc.tile_pool(name="ps2p", bufs=3, space="PSUM") as ps2p,
        ):
            # --- PE warm-up: matmuls on a zeroed tile, no DMA deps. The tail
            # of the chain uses full N=512 matmuls so the HAM activity monitor
            # sees solid PE-busy windows right before the real stream starts.
            # memset on DVE so the ACT engine's one-time Gelu table load can
            # happen concurrently (and isn't preceded by a Copy-table load).
            warm = wpool.tile([P, TT], BF16, name="warm", tag="warm")
            nc.vector.memset(warm, 0.0)
            ps_w = ps1p.tile([P, TT], F32, name="ps_w", tag="ps1")
            for _ in range(WARMUP_MMS):
                nc.tensor.matmul(ps_w[:64, :64], warm[:, :64], warm[:, :64], start=True, stop=True)
            for _ in range(WARMUP_BIG_MMS):
                nc.tensor.matmul(ps_w[:64, :], warm[:, :64], warm, start=True, stop=True)

            # --- Weights + all of x resident in SBUF (bf16: 74KB/partition).
            w1_sb = wpool.tile([P, E_LOC, N_DT, H], BF16, name="w1_sb", tag="w1")
            w2_sb = wpool.tile([P, E_LOC, N_HT, D4], BF16, name="w2_sb", tag="w2")
            x_sb = wpool.tile([P, N_DT, NTOK], BF16, name="x_sb", tag="x")

            # Input stream on SP, few big DMAs, first-needed first:
            # w1[e0] + x(t0) unlock the first matmul group (~11.5us),
            # w1[e1] is needed ~3.5us later, the w2s only at the first mm2.
            nc.sync.dma_start(w1_sb[:, 0], w1[:, 0])
            tok0 = slice(0, TT)
            for dt in range(N_DT):
                nc.sync.dma_start(x_sb[:, dt, tok0], xT[:, dt, tok0])
            for e in range(1, E_LOC):
                nc.sync.dma_start(w1_sb[:, e], w1[:, e])
            for e in range(E_LOC):
                nc.sync.dma_start(w2_sb[:, e], w2[:, e])
            for t0, t1 in ((1, 2), (2, 3), (3, 4), (4, 8), (8, 12), (12, 16)):
                tok = slice(t0 * TT, t1 * TT)
                nc.sync.dma_start(x_sb[:, :, tok], xT[:, :, tok])

            for t in range(NTOK // TT):
                tok = slice(t * TT, (t + 1) * TT)
                hT_tiles = []
                for e in range(E_LOC):
                    hT_sb = hpool.tile([P, N_HT, TT], BF16, name="hT_sb", tag="h")
                    for ht in range(N_HT):
                        ps1 = ps1p.tile([P, TT], F32, name="ps1", tag="ps1")
                        for dt in range(N_DT):
                            nc.tensor.matmul(
                                ps1,
                                w1_sb[:, e, dt, ht * P : (ht + 1) * P],
                                x_sb[:, dt, tok],
                                start=(dt == 0),
                                stop=(dt == N_DT - 1),
                            )
                        nc.scalar.activation(hT_sb[:, ht, :], ps1, gelu)
                    hT_tiles.append(hT_sb)
                # The last token tile runs mm2/cast/DMA in two 256-token
                # halves so the final output DMA starts ~2us earlier.
                halves = 2 if t == NTOK // TT - 1 else 1
                hw = TT // halves
                for h in range(halves):
                    hsl = slice(h * hw, (h + 1) * hw)
                    o_sb = opool.tile([P, E_LOC, hw], BF16, name="o_sb", tag="o")
                    for e in range(E_LOC):
                        ps2 = ps2p.tile([P, hw], F32, name="ps2", tag="ps2")
                        for ht in range(N_HT):
                            nc.tensor.matmul(
                                ps2,
                                w2_sb[:, e, ht, :],
                                hT_tiles[e][:, ht, hsl],
                                start=(ht == 0),
                                stop=(ht == N_HT - 1),
                            )
                        nc.vector.tensor_copy(o_sb[:, e, :], ps2)
                    # Output DMA from the Activation engine's HW-DGE queue
                    # (SP-issued DMAs mid-kernel congest the semaphore path).
                    htok = slice(t * TT + h * hw, t * TT + (h + 1) * hw)
                    nc.scalar.dma_start(outT[:, :, htok], o_sb)

    nc.finalize()
    return nc


_NC = None


def _get_program():
    global _NC
    if _NC is None:
        _NC = _build_program()
    return _NC


def _prepare_in_maps(x: np.ndarray, w1: np.ndarray, w2: np.ndarray):
    """Host-side swizzle + bf16 cast into per-core input maps."""
    # xT[p, dt, n] = x[n, dt*128 + p]
    xT = (
        x.reshape(NTOK, N_DT, P)
        .transpose(2, 1, 0)
        .astype(ml_dtypes.bfloat16)
    )
    xT = np.ascontiguousarray(xT)
    in_maps = []
    for c in range(N_CORES):
        w1c = w1[c * E_LOC : (c + 1) * E_LOC]  # [e, d, h]
        w2c = w2[c * E_LOC : (c + 1) * E_LOC]  # [e, h, d4]
        # w1_dr[p, e, dt, h] = w1c[e, dt*128+p, h]
        w1d = np.ascontiguousarray(
            w1c.reshape(E_LOC, N_DT, P, H).transpose(2, 0, 1, 3)
        ).astype(ml_dtypes.bfloat16)
        w2d = np.ascontiguousarray(
            w2c.reshape(E_LOC, N_HT, P, D4).transpose(2, 0, 1, 3)
        ).astype(ml_dtypes.bfloat16)
        in_maps.append({"xT": xT, "w1": w1d, "w2": w2d})
    return in_maps


def kernel(x: np.ndarray, w1: np.ndarray, w2: np.ndarray, **_) -> np.ndarray:
    """Full inputs in, full output out; expert-parallel across 8 NeuronCores."""
    nc = _get_program()
    in_maps = _prepare_in_maps(x, w1, w2)
    res = run_bass_kernel_spmd(nc, in_maps, list(range(N_CORES)))

    # res outT: [d4, e_loc, tok] per core -> out[n, d4, e] with e = c*E_LOC+el
    full = np.stack([res.results[c]["outT"] for c in range(N_CORES)], axis=0)
    out = full.transpose(3, 1, 0, 2).astype(np.float32)  # [tok, d4, core, e_loc]
    return np.ascontiguousarray(out.reshape(4, 2048, D4, E))
